# revision 3
# baseline (speedup 1.0000x reference)
"""Causal depthwise conv1d (B=4, T=8192, F=1024, K=4) on 8 trn2 NeuronCores.

Sharding: feature dim F split 8 ways (128 channels/core, no communication).
Host side transposes each shard to channel-major (128, B*T) so every DMA is
contiguous per partition. On-core layout: partition = channel, free dim = time.

Per tile (TCOLS time steps + 3-col left halo), out[:, t] = sum_k w_k*x[t+k-3] + b:
    ACT:  tm = Identity(x0 * w0 + bias)      per-partition scale/bias
    DVE:  tm = (x1 * w1) + tm                scalar_tensor_tensor (fused MAC)
    ACT:  d  = Copy(x2 * w2)
    GPS:  tm = tm + d                        tensor_add (TensorScalarPtr is not
                                             a Pool opcode, plain add is)
    DVE:  out = (x3 * w3) + tm
All DMAs are HWDGE (nc.sync) so they never contend with compute engines.
"""

import numpy as np
from contextlib import ExitStack

import concourse.bacc as bacc
import concourse.tile as tile
from concourse import mybir
from concourse.bass_utils import run_bass_kernel_spmd

B, T, F, K = 4, 8192, 1024, 4
N_CORES = 8
CPC = F // N_CORES  # 128 channels per core

F32 = mybir.dt.float32


def _build_nc(
    n_segs: int, seg_cols: int, tiles_per_seg: int, use_gpsimd: bool = True
):
    nc = bacc.Bacc(
        "TRN2", target_bir_lowering=False, debug=False, num_devices=N_CORES
    )
    tot = n_segs * seg_cols
    tcols = seg_cols // tiles_per_seg
    assert seg_cols % tiles_per_seg == 0

    x_d = nc.dram_tensor("x", [CPC, tot], F32, kind="ExternalInput").ap()
    w_d = nc.dram_tensor("w", [CPC, K], F32, kind="ExternalInput").ap()
    b_d = nc.dram_tensor("b", [CPC, 1], F32, kind="ExternalInput").ap()
    o_d = nc.dram_tensor("out", [CPC, tot], F32, kind="ExternalOutput").ap()

    mult = mybir.AluOpType.mult
    add = mybir.AluOpType.add
    H = K - 1  # halo

    with tile.TileContext(nc) as tc, ExitStack() as ctx:
        cpool = ctx.enter_context(tc.tile_pool(name="consts", bufs=1))
        w_sb = cpool.tile([CPC, K], F32)
        b_sb = cpool.tile([CPC, 1], F32)
        nc.sync.dma_start(out=w_sb[:], in_=w_d[:, :])
        nc.sync.dma_start(out=b_sb[:], in_=b_d[:, :])

        xp = ctx.enter_context(tc.tile_pool(name="xp", bufs=2))
        tp = ctx.enter_context(tc.tile_pool(name="tp", bufs=2))
        dp = ctx.enter_context(tc.tile_pool(name="dp", bufs=2))
        op = ctx.enter_context(tc.tile_pool(name="op", bufs=2))

        for s in range(n_segs):
            for j in range(tiles_per_seg):
                t0 = s * seg_cols + j * tcols
                xt = xp.tile([CPC, tcols + H], F32)
                if j == 0:
                    # batch start: zero halo
                    nc.vector.memset(xt[:, 0:H], 0.0)
                    nc.sync.dma_start(
                        out=xt[:, H:], in_=x_d[:, t0 : t0 + tcols]
                    )
                else:
                    nc.sync.dma_start(
                        out=xt[:], in_=x_d[:, t0 - H : t0 + tcols]
                    )

                tm = tp.tile([CPC, tcols], F32)
                nc.scalar.activation(
                    tm[:],
                    xt[:, 0:tcols],
                    mybir.ActivationFunctionType.Identity,
                    bias=b_sb[:],
                    scale=w_sb[:, 0:1],
                )
                nc.vector.scalar_tensor_tensor(
                    tm[:], xt[:, 1 : 1 + tcols], w_sb[:, 1:2], tm[:], mult, add
                )
                if use_gpsimd:
                    d = dp.tile([CPC, tcols], F32)
                    nc.scalar.activation(
                        d[:],
                        xt[:, 2 : 2 + tcols],
                        mybir.ActivationFunctionType.Copy,
                        bias=0.0,
                        scale=w_sb[:, 2:3],
                    )
                    nc.gpsimd.tensor_add(tm[:], tm[:], d[:])
                else:
                    nc.vector.scalar_tensor_tensor(
                        tm[:],
                        xt[:, 2 : 2 + tcols],
                        w_sb[:, 2:3],
                        tm[:],
                        mult,
                        add,
                    )
                ot = op.tile([CPC, tcols], F32)
                nc.vector.scalar_tensor_tensor(
                    ot[:], xt[:, 3 : 3 + tcols], w_sb[:, 3:4], tm[:], mult, add
                )
                nc.sync.dma_start(out=o_d[:, t0 : t0 + tcols], in_=ot[:])

    nc.compile()
    return nc


def _shard_inputs(x: np.ndarray, w: np.ndarray, b: np.ndarray):
    # x: (B, T, F) -> channel-major (F, B*T), then split along channels.
    xs = np.ascontiguousarray(np.transpose(x, (2, 0, 1)).reshape(F, B * T))
    in_maps = []
    for c in range(N_CORES):
        sl = slice(c * CPC, (c + 1) * CPC)
        in_maps.append(
            {
                "x": np.ascontiguousarray(xs[sl]),
                "w": np.ascontiguousarray(w[:, 0, sl].T),
                "b": np.ascontiguousarray(b[sl].reshape(CPC, 1)),
            }
        )
    return in_maps


def _unshard_output(results) -> np.ndarray:
    out = np.empty((B, T, F), np.float32)
    for c in range(N_CORES):
        oc = results[c]["out"]  # (CPC, B*T)
        out[:, :, c * CPC : (c + 1) * CPC] = oc.reshape(CPC, B, T).transpose(
            1, 2, 0
        )
    return out


def _run(
    x,
    w,
    b,
    trace: bool = False,
    use_gpsimd: bool = True,
    tiles_per_seg: int = 2,
    tmpdir=None,
):
    x = np.asarray(x, dtype=np.float32)
    w = np.asarray(w, dtype=np.float32)
    b = np.asarray(b, dtype=np.float32)
    in_maps = _shard_inputs(x, w, b)
    nc = _build_nc(B, T, tiles_per_seg, use_gpsimd=use_gpsimd)
    br = run_bass_kernel_spmd(
        nc, in_maps, core_ids=list(range(N_CORES)), trace=trace, tmpdir=tmpdir
    )
    return _unshard_output(br.results), br


def kernel(x, w, b):
    out, _ = _run(x, w, b, trace=False)
    return out


# revision 10
# speedup vs baseline: 1.7551x; 1.7551x over previous
"""Causal depthwise conv1d (B=4, T=8192, F=1024, K=4) on 8 trn2 NeuronCores.

Sharding: feature dim F split 8 ways (128 channels/core, no communication).
Host side transposes each shard to channel-major (128, B*T) so every DMA is
contiguous per partition. On-core layout: partition = channel, free dim = time.

Per tile (tcols time steps + 3-col left halo), out[:, t] = sum_k w_k*x[t+k-3] + b.
Columns are split between two compute paths that run in parallel:

  PE path (pe_chunks x 512 cols): psum = sum_k diag(w_k) @ x_k, 4 fp32 matmuls
      accumulating in one PSUM bank (contraction over the channel partition
      picks out channel m: out[m,n] = w_m * x[m,n]). ACT evacuates PSUM->SBUF
      with the bias via activation(Identity, bias).

  DVE path (remaining cols): shallow tree
      ACT: tm = Identity(x0*w0 + bias); d = Copy(x2*w2)
      DVE: tm = (x1*w1) + tm; d = (x3*w3) + d   (scalar_tensor_tensor MACs)
      DVE: out = tm + d

GpSimd is deliberately unused: any Pool elementwise op contends with DVE's
second SBUF port (measured 3x mutual slowdown). All DMAs are HWDGE (nc.sync).
"""

import numpy as np
from contextlib import ExitStack

import concourse.bacc as bacc
import concourse.tile as tile
from concourse import mybir
from concourse.bass_utils import run_bass_kernel_spmd

B, T, F, K = 4, 8192, 1024, 4
N_CORES = 8
CPC = F // N_CORES  # 128 channels per core

F32 = mybir.dt.float32
MM_N = 512  # fp32 moving-operand max free dim = one PSUM bank


def _build_nc(
    n_segs: int,
    seg_cols: int,
    tiles_per_seg: int,
    pe_chunks: int = 3,
):
    nc = bacc.Bacc(
        "TRN2", target_bir_lowering=False, debug=False, num_devices=N_CORES
    )
    tot = n_segs * seg_cols
    tcols = seg_cols // tiles_per_seg
    assert seg_cols % tiles_per_seg == 0
    pe_cols = pe_chunks * MM_N
    dve_cols = tcols - pe_cols
    assert 0 <= pe_cols <= tcols

    x_d = nc.dram_tensor("x", [CPC, tot], F32, kind="ExternalInput").ap()
    w_d = nc.dram_tensor("w", [CPC, K], F32, kind="ExternalInput").ap()
    b_d = nc.dram_tensor("b", [CPC, 1], F32, kind="ExternalInput").ap()
    if pe_chunks > 0:
        dw_d = nc.dram_tensor(
            "dw", [K, CPC, CPC], F32, kind="ExternalInput"
        ).ap()
    o_d = nc.dram_tensor("out", [CPC, tot], F32, kind="ExternalOutput").ap()

    mult = mybir.AluOpType.mult
    add = mybir.AluOpType.add
    ident = mybir.ActivationFunctionType.Identity
    copyf = mybir.ActivationFunctionType.Copy
    H = K - 1  # halo

    with tile.TileContext(nc) as tc, ExitStack() as ctx:
        cpool = ctx.enter_context(tc.tile_pool(name="consts", bufs=1))
        w_sb = cpool.tile([CPC, K], F32)
        b_sb = cpool.tile([CPC, 1], F32)
        nc.sync.dma_start(out=w_sb[:], in_=w_d[:, :])
        nc.sync.dma_start(out=b_sb[:], in_=b_d[:, :])
        if pe_chunks > 0:
            dw_sb = [
                cpool.tile([CPC, CPC], F32, name=f"dw{k}", tag=f"dw{k}")
                for k in range(K)
            ]
            for k in range(K):
                nc.sync.dma_start(out=dw_sb[k][:], in_=dw_d[k])

        xp = ctx.enter_context(tc.tile_pool(name="xp", bufs=3))
        op = ctx.enter_context(tc.tile_pool(name="op", bufs=3))
        if dve_cols > 0:
            tp = ctx.enter_context(tc.tile_pool(name="tp", bufs=3))
            dp = ctx.enter_context(tc.tile_pool(name="dp", bufs=3))
        if pe_chunks > 0:
            pp = ctx.enter_context(
                tc.tile_pool(name="pp", bufs=4, space="PSUM")
            )

        for s in range(n_segs):
            for j in range(tiles_per_seg):
                t0 = s * seg_cols + j * tcols
                xt = xp.tile([CPC, tcols + H], F32)
                if j == 0:
                    # batch start: zero halo
                    nc.vector.memset(xt[:, 0:H], 0.0)
                    nc.sync.dma_start(
                        out=xt[:, H:], in_=x_d[:, t0 : t0 + tcols]
                    )
                else:
                    nc.sync.dma_start(
                        out=xt[:], in_=x_d[:, t0 - H : t0 + tcols]
                    )

                ot = op.tile([CPC, tcols], F32)

                # --- PE path ---
                for c in range(pe_chunks):
                    c0 = c * MM_N
                    ps = pp.tile([CPC, MM_N], F32)
                    for k in range(K):
                        nc.tensor.matmul(
                            ps[:],
                            dw_sb[k][:],
                            xt[:, k + c0 : k + c0 + MM_N],
                            start=(k == 0),
                            stop=(k == K - 1),
                        )
                    nc.scalar.activation(
                        ot[:, c0 : c0 + MM_N],
                        ps[:],
                        ident,
                        bias=b_sb[:],
                        scale=1.0,
                    )

                # --- DVE path ---
                if dve_cols > 0:
                    q = pe_cols  # output column offset of the DVE range
                    tm = tp.tile([CPC, dve_cols], F32)
                    nc.scalar.activation(
                        tm[:],
                        xt[:, q : q + dve_cols],
                        ident,
                        bias=b_sb[:],
                        scale=w_sb[:, 0:1],
                    )
                    nc.vector.scalar_tensor_tensor(
                        tm[:],
                        xt[:, q + 1 : q + 1 + dve_cols],
                        w_sb[:, 1:2],
                        tm[:],
                        mult,
                        add,
                    )
                    d = dp.tile([CPC, dve_cols], F32)
                    nc.scalar.activation(
                        d[:],
                        xt[:, q + 2 : q + 2 + dve_cols],
                        copyf,
                        bias=0.0,
                        scale=w_sb[:, 2:3],
                    )
                    nc.vector.scalar_tensor_tensor(
                        d[:],
                        xt[:, q + 3 : q + 3 + dve_cols],
                        w_sb[:, 3:4],
                        d[:],
                        mult,
                        add,
                    )
                    nc.vector.tensor_add(ot[:, q:], tm[:], d[:])

                nc.sync.dma_start(out=o_d[:, t0 : t0 + tcols], in_=ot[:])

    nc.compile()
    return nc


def _shard_inputs(x, w, b, pe_chunks: int):
    # x: (B, T, F) -> channel-major (F, B*T), then split along channels.
    xs = np.ascontiguousarray(np.transpose(x, (2, 0, 1)).reshape(F, B * T))
    in_maps = []
    for c in range(N_CORES):
        sl = slice(c * CPC, (c + 1) * CPC)
        wc = np.ascontiguousarray(w[:, 0, sl])  # (K, CPC)
        m = {
            "x": np.ascontiguousarray(xs[sl]),
            "w": np.ascontiguousarray(wc.T),
            "b": np.ascontiguousarray(b[sl].reshape(CPC, 1)),
        }
        if pe_chunks > 0:
            dw = np.zeros((K, CPC, CPC), np.float32)
            for k in range(K):
                np.fill_diagonal(dw[k], wc[k])
            m["dw"] = dw
        in_maps.append(m)
    return in_maps


def _unshard_output(results) -> np.ndarray:
    out = np.empty((B, T, F), np.float32)
    for c in range(N_CORES):
        oc = results[c]["out"]  # (CPC, B*T)
        out[:, :, c * CPC : (c + 1) * CPC] = oc.reshape(CPC, B, T).transpose(
            1, 2, 0
        )
    return out


def _run(
    x,
    w,
    b,
    trace: bool = False,
    tiles_per_seg: int = 2,
    pe_chunks: int = 3,
    tmpdir=None,
):
    x = np.asarray(x, dtype=np.float32)
    w = np.asarray(w, dtype=np.float32)
    b = np.asarray(b, dtype=np.float32)
    in_maps = _shard_inputs(x, w, b, pe_chunks)
    nc = _build_nc(B, T, tiles_per_seg, pe_chunks=pe_chunks)
    br = run_bass_kernel_spmd(
        nc, in_maps, core_ids=list(range(N_CORES)), trace=trace, tmpdir=tmpdir
    )
    return _unshard_output(br.results), br


def kernel(x, w, b):
    out, _ = _run(x, w, b, trace=False)
    return out
